# revision 25
# baseline (speedup 1.0000x reference)
"""Causal attention (weight-normalized projections) Trainium2 Bass kernel.

Full-input contract: kernel(**inputs) takes the unsharded tensors from
setup_inputs() and returns the full [8, 32, 32, 512] output. Internally the
batch dim (8) is sharded 1:1 across 8 NeuronCores (data parallel); each core
runs an identical Bass program on its own batch.

Math per batch b:
  qf = query[b].reshape(1024, 256); kf = key[b].reshape(1024, 512)
  q = qf @ wq + bq ; k = kf @ wk + bk ; v = kf @ wv + bv      (wx weight-normed)
  per head h (8 heads, dh=64):
    scores = q_h @ k_h.T / 8 ; strict-causal mask ; softmax ; out_h = attn @ v_h
  out[b] = concat_h(out_h).reshape(32, 32, 512)

Numerics: softmax is computed without max-subtraction (scores are ~N(0,1);
exp never overflows fp32). The mask is applied multiplicatively after exp
(0/1 mask), which matches the reference's -10000 additive mask exactly in
fp32 (exp underflows to 0). Row q=0 has an all-zero mask; its numerator and
denominator are exactly 0 and the 1e-30 epsilon makes 0/eps = 0, matching
the reference's post-softmax start-mask zeroing.
"""

import os
import sys

import numpy as np

for _p in ("/opt/trn_rl_repo", "/root/.axon_site/_ro/trn_rl_repo"):
    if _p not in sys.path and os.path.isdir(_p):
        sys.path.append(_p)

import concourse.bass as bass
import concourse.mybir as mybir
import concourse.tile as tile

FP = mybir.dt.float32
FPR = mybir.dt.float32r
BF = mybir.dt.bfloat16
AF = mybir.ActivationFunctionType


B = 8
S = 1024
QC, KC, CH = 256, 512, 512
NH, DH = 8, 64
P = 128
NS = S // P    # 8 seq chunks of 128
NAQ = QC // P  # 2 contraction chunks for q proj
NAK = KC // P  # 4 contraction chunks for k/v proj
NCC = CH // P  # 4 output-channel chunks
DH1 = DH + 1   # v columns + ones column (softmax denominator)

N_CORES = 8

_cached_nc = None


def _split_multi_waits(nc, engines=("PE",)):
    """Hoist extra sem-waits onto single-wait NoOps.

    Walrus's CoreV3 codegen rejects PE instructions carrying more than one
    sync wait (setupSyncWait<S3_LW_STRUCT>: "Too many sync wait commands").
    Tile's scheduler freely attaches several waits to one instruction, so
    after scheduling we move all but the last wait of each affected
    instruction onto dedicated same-engine NoOps placed directly before it;
    the engine's sequencer blocks on each NoOp in program order, preserving
    semantics exactly.
    """
    ctr = 0
    for fn in nc.m.functions:
        for blk in fn.blocks:
            new_insts = []
            for inst in blk.instructions:
                si = getattr(inst, "sync_info", None)
                waits = list(si.on_wait) if si is not None and si.on_wait else []
                eng = getattr(inst, "engine", None)
                if (
                    len(waits) > 1
                    and eng is not None
                    and any(e in str(eng) for e in engines)
                ):
                    for w in waits[:-1]:
                        nop = mybir.InstNoOp(
                            name=f"I-wsplit-{ctr}",
                            engine=eng,
                            sync_info=mybir.SyncInfo(on_wait=[w], on_update=[]),
                            bass_nofuse=True,
                        )
                        ctr += 1
                        new_insts.append(nop)
                        nc.inst_map[nop.name] = nop
                    inst.sync_info = mybir.SyncInfo(
                        on_wait=[waits[-1]],
                        on_update=list(si.on_update) if si.on_update else [],
                    )
                new_insts.append(inst)
            blk.instructions[:] = new_insts


def build_module() -> "bass.Bass":
    nc = bass.Bass()

    qf_d = nc.dram_tensor("qf", [S, QC], BF, kind="ExternalInput")
    kf_d = nc.dram_tensor("kf", [S, KC], BF, kind="ExternalInput")
    wq_d = nc.dram_tensor("wq", [QC, CH], BF, kind="ExternalInput")
    wk_d = nc.dram_tensor("wk", [KC, CH], BF, kind="ExternalInput")
    wv_d = nc.dram_tensor("wv", [KC, CH], BF, kind="ExternalInput")
    bq_d = nc.dram_tensor("bq", [P, NCC], FP, kind="ExternalInput")
    bk_d = nc.dram_tensor("bk", [P, NCC], FP, kind="ExternalInput")
    bvb_d = nc.dram_tensor("bvb", [P, CH], BF, kind="ExternalInput")
    mask_d = nc.dram_tensor("maskT", [P, P], BF, kind="ExternalInput")
    id_d = nc.dram_tensor("ident", [P, P], FP, kind="ExternalInput")
    idb_d = nc.dram_tensor("identb", [P, P], BF, kind="ExternalInput")
    ones_d = nc.dram_tensor("ones", [1, P], BF, kind="ExternalInput")
    bvr_d = nc.dram_tensor("bvr", [1, CH], BF, kind="ExternalInput")
    vones_d = nc.dram_tensor("vones", [P, NH], BF, kind="ExternalInput")
    out_d = nc.dram_tensor("out", [S, CH], FP, kind="ExternalOutput")

    QW = 512  # q-half width

    with tile.TileContext(nc) as tc:
        with (
            tc.tile_pool(name="const", bufs=1) as cpool,
            tc.tile_pool(name="work", bufs=2) as wpool,
            tc.tile_pool(name="psA", bufs=2, space=bass.MemorySpace.PSUM) as psA,
            tc.tile_pool(name="psB", bufs=3, space=bass.MemorySpace.PSUM) as psB,
            tc.tile_pool(name="psC", bufs=1, space=bass.MemorySpace.PSUM) as psC,
        ):
            # ---- inputs: natural contiguous loads; transpose on (idle) PE ----
            qfT = [cpool.tile([P, S], BF, tag=f"qfT{a}", name=f"qfT{a}") for a in range(NAQ)]
            kfT = [cpool.tile([P, S], BF, tag=f"kfT{a}", name=f"kfT{a}") for a in range(NAK)]
            wq_sb = [cpool.tile([P, CH], BF, tag=f"wq{a}", name=f"wq{a}") for a in range(NAQ)]
            wk_sb = [cpool.tile([P, CH], BF, tag=f"wk{a}", name=f"wk{a}") for a in range(NAK)]
            wv_sb = [cpool.tile([P, CH], BF, tag=f"wv{a}", name=f"wv{a}") for a in range(NAK)]
            idb_sb = cpool.tile([P, P], BF, tag="identb", name="idb_sb")
            nc.sync.dma_start(idb_sb[:], idb_d[:])
            # PE warm-up: dense dummy matmuls during the input-DMA window keep
            # the HAM activity monitor busy so projections start at 2.4 GHz
            # instead of the cold 1.2 GHz half-clock.
            warm_ps = psC.tile([P, QW], FP, tag="tp", name="warm_ps")
            for _w in range(56):
                nc.tensor.matmul(
                    warm_ps[0:P, 0:P], idb_sb[:], idb_sb[:],
                    start=True, stop=True,
                )
            qf_sb = [cpool.tile([P, QC], BF, tag=f"qf{si}", name=f"qf{si}") for si in range(NS)]
            kf_sb = [cpool.tile([P, KC], BF, tag=f"kf{si}", name=f"kf{si}") for si in range(NS)]
            for si in range(NS):
                nc.sync.dma_start(qf_sb[si][:], qf_d[si * P:(si + 1) * P, :])
            for a in range(NAQ):
                nc.sync.dma_start(wq_sb[a][:], wq_d[a * P:(a + 1) * P, :])
            for si in range(NS):
                nc.sync.dma_start(kf_sb[si][:], kf_d[si * P:(si + 1) * P, :])
            for a in range(NAK):
                nc.sync.dma_start(wk_sb[a][:], wk_d[a * P:(a + 1) * P, :])
                nc.sync.dma_start(wv_sb[a][:], wv_d[a * P:(a + 1) * P, :])
            bq_sb = cpool.tile([P, NCC], FP, tag="bq", name="bq_sb")
            bk_sb = cpool.tile([P, NCC], FP, tag="bk", name="bk_sb")
            bvb_sb = cpool.tile([P, CH], BF, tag="bvb", name="bvb_sb")
            ones_sb = cpool.tile([1, P], BF, tag="ones", name="ones_sb")
            bvr_sb = cpool.tile([1, CH], BF, tag="bvr", name="bvr_sb")
            nc.sync.dma_start(ones_sb[:], ones_d[:])
            nc.sync.dma_start(bvr_sb[:], bvr_d[:])
            mask_sb = cpool.tile([P, P], BF, tag="mask", name="mask_sb")
            id_sb = cpool.tile([P, P], FP, tag="ident", name="id_sb")
            nc.sync.dma_start(bq_sb[:], bq_d[:])
            nc.sync.dma_start(bk_sb[:], bk_d[:])
            nc.sync.dma_start(bvb_sb[:], bvb_d[:])
            nc.sync.dma_start(mask_sb[:], mask_d[:])
            nc.sync.dma_start(id_sb[:], id_d[:])

            for a in range(NAQ):
                ps = psA.tile([P, S], FP, tag="sc", name="sc_ps")
                for si in range(NS):
                    nc.tensor.matmul(
                        ps[:, si * P:(si + 1) * P],
                        qf_sb[si][:, a * P:(a + 1) * P],
                        idb_sb[:],
                        start=True,
                        stop=True,
                    )
                if a % 2 == 0:
                    nc.vector.tensor_copy(qfT[a][:], ps[:])
                else:
                    nc.scalar.copy(qfT[a][:], ps[:])
            for a in range(NAK):
                ps = psA.tile([P, S], FP, tag="sc", name="sc_ps")
                for si in range(NS):
                    nc.tensor.matmul(
                        ps[:, si * P:(si + 1) * P],
                        kf_sb[si][:, a * P:(a + 1) * P],
                        idb_sb[:],
                        start=True,
                        stop=True,
                    )
                if a % 2 == 0:
                    nc.vector.tensor_copy(kfT[a][:], ps[:])
                else:
                    nc.scalar.copy(kfT[a][:], ps[:])

            # ---------------- projections ----------------
            # qT/kT in [channel, seq] layout (head-dim on partitions)
            qT = [cpool.tile([P, S], BF, tag=f"qT{c}", name=f"qT{c}") for c in range(NCC)]
            kT = [cpool.tile([P, S], BF, tag=f"kT{c}", name=f"kT{c}") for c in range(NCC)]

            def emit_qT(c):
                ps = psA.tile([P, S], FP, tag="sc", name="sc_ps")
                for a in range(NAQ):
                    for g in range(2):
                        nc.tensor.matmul(
                            ps[:, g * QW:(g + 1) * QW],
                            wq_sb[a][:, c * P:(c + 1) * P],
                            qfT[a][:, g * QW:(g + 1) * QW],
                            start=(a == 0),
                            stop=(a == NAQ - 1),
                        )
                nc.scalar.activation(
                    qT[c][:], ps[:], AF.Identity, bias=bq_sb[:, c:c + 1]
                )

            def emit_kT(c):
                ps = psA.tile([P, S], FP, tag="sc", name="sc_ps")
                for a in range(NAK):
                    for g in range(2):
                        nc.tensor.matmul(
                            ps[:, g * QW:(g + 1) * QW],
                            wk_sb[a][:, c * P:(c + 1) * P],
                            kfT[a][:, g * QW:(g + 1) * QW],
                            start=(a == 0),
                            stop=(a == NAK - 1),
                        )
                nc.scalar.activation(
                    kT[c][:], ps[:], AF.Identity, bias=bk_sb[:, c:c + 1]
                )

            # v[s, c] per-head blocks of 65 cols (64 data + ones col for the
            # softmax denominator); bias added on DVE during evacuation
            v_sb = [cpool.tile([P, NH * DH1], BF, tag=f"v{si}", name=f"v{si}") for si in range(NS)]
            bvb_view = bvb_sb[:].rearrange("p (h d) -> p h d", h=NH)

            def emit_v(si):
                ps = psA.tile([P, S], FP, tag="sc", name="sc_ps")
                for a in range(NAK):
                    nc.tensor.matmul(
                        ps[:, 0:CH],
                        kfT[a][:, si * P:(si + 1) * P],
                        wv_sb[a][:],
                        start=(a == 0),
                        stop=False,
                    )
                nc.tensor.matmul(
                    ps[:, 0:CH], ones_sb[:], bvr_sb[:], start=False, stop=True
                )
                v_view = v_sb[si][:].rearrange("p (h d) -> p h d", h=NH)
                nc.vector.tensor_copy(
                    v_view[:, :, 0:DH],
                    ps[:, 0:CH].rearrange("p (h d) -> p h d", h=NH),
                )
                nc.sync.dma_start(
                    v_view[:, :, DH:DH1],
                    vones_d[:].rearrange("p (h o) -> p h o", o=1),
                )

            for c in range(NCC):
                emit_qT(c)
            for c in range(NCC):
                emit_kT(c)
            for si in range(NS):
                emit_v(si)

            # ---------------- attention: head pairs x q-halves ----------------
            # Heads 2p/2p+1 share qT[p]/kT[p] (rows 0:64 / 64:128). QK for the
            # two heads is row-packed onto the PE array (tile_position), the
            # exp over both heads' scores is one ACT instruction, and the two
            # AV chains interleave to keep PE fed while ACT runs.
            out_sb = cpool.tile([P, NS * CH], FP, tag="osb", name="out_sb")
            mask_b2 = mask_sb[:].rearrange("p (o w) -> p o w", o=1).broadcast_to((P, 2, P))

            for p in range(NH // 2):
                tq = qT[p]
                tk = kT[p]
                v_hp = [
                    [v_sb[j][:].rearrange("p (h d) -> p h d", h=NH)[:, 2 * p + idx, :]
                     for idx in range(2)]
                    for j in range(NS)
                ]
                for g in range(2):
                    jmax = 4 * (g + 1)
                    outp = [
                        psB.tile([P, QW], FP, tag="outp", name="outp_ps")
                        for _ in range(2)
                    ]

                    def emit_qk(j):
                        off = max(0, j * P - g * QW)
                        sc = psA.tile([P, 2 * QW], FP, tag="sc", name="sc_ps")
                        for idx in range(2):
                            nc.tensor.matmul(
                                sc[:, idx * QW + off:(idx + 1) * QW],
                                tk[idx * DH:(idx + 1) * DH, j * P:(j + 1) * P],
                                tq[idx * DH:(idx + 1) * DH, g * QW + off:(g + 1) * QW],
                                start=True,
                                stop=True,
                                tile_position=(idx * DH, 0),
                            )
                        ex = wpool.tile([P, 2 * QW], BF, tag="ex", name="ex_t", bufs=3)
                        scv = sc[:].rearrange("p (i w) -> p i w", i=2)[:, :, off:QW]
                        exv = ex[:].rearrange("p (i w) -> p i w", i=2)[:, :, off:QW]
                        nc.scalar.activation(exv, scv, AF.Exp, scale=0.125)
                        if g * 4 <= j < g * 4 + 4:  # diagonal block in this half
                            od = j * P - g * QW
                            exd = ex[:].rearrange("p (i w) -> p i w", i=2)[:, :, od:od + P]
                            if j % 2 == 0:
                                nc.vector.tensor_mul(exd, exd, mask_b2)
                            else:
                                nc.gpsimd.tensor_mul(exd, exd, mask_b2)
                        return ex

                    def emit_av(j, ex):
                        off = max(0, j * P - g * QW)
                        for idx in range(2):
                            nc.tensor.matmul(
                                outp[idx][0:DH1, off:QW],
                                v_hp[j][idx],
                                ex[:, idx * QW + off:(idx + 1) * QW],
                                start=(j == 0),
                                stop=(j == jmax - 1),
                                skip_group_check=True,
                            )

                    prev_ex = emit_qk(0)
                    for j in range(1, jmax):
                        cur_ex = emit_qk(j)
                        emit_av(j - 1, prev_ex)
                        prev_ex = cur_ex
                    emit_av(jmax - 1, prev_ex)

                    # epilogue per head: evac, transpose to [q, d], normalize
                    for idx in range(2):
                        h = 2 * p + idx
                        outs = wpool.tile([P, QW], FP, tag="outs", name="outs_t")
                        if idx == 0:
                            nc.vector.tensor_copy(outs[0:DH1, :], outp[idx][0:DH1, :])
                        else:
                            nc.scalar.copy(outs[0:DH1, :], outp[idx][0:DH1, :])
                        tp2 = psC.tile([P, QW], FP, tag="tp", name="tp_ps")
                        for ls in range(4):
                            nc.tensor.transpose(
                                tp2[:, ls * P:ls * P + DH1],
                                outs[0:DH1, ls * P:(ls + 1) * P],
                                id_sb[0:DH1, 0:DH1],
                            )
                        tpv = tp2[:].rearrange("p (s c) -> p s c", c=P)
                        rc = wpool.tile([P, 4], FP, tag="rc", name="rc_t")
                        rc2 = wpool.tile([P, 4], FP, tag="rc2", name="rc2_t")
                        nc.vector.tensor_scalar_add(rc[:], tpv[:, :, DH:DH1], 1e-30)
                        nc.vector.reciprocal(rc2[:], rc[:])
                        out_view = out_sb[:].rearrange(
                            "p (s h d) -> p s h d", s=NS, h=NH
                        )[:, 4 * g:4 * (g + 1), h, :]
                        rc_b = rc2[:].rearrange("p (s o) -> p s o", o=1).broadcast_to(
                            (P, 4, DH)
                        )
                        nc.vector.tensor_mul(out_view, tpv[:, :, 0:DH], rc_b)

                    if p == 0 and g == 0:
                        # row q=0 is exactly zero (start mask); write before
                        # the first partial output DMA of chunk (p=0, g=0)
                        nc.vector.memset(
                            out_sb[0:1, 0:2 * DH], 0.0
                        )
                    nc.sync.dma_start(
                        out_d.rearrange("(s p) c -> p s c", p=P)[
                            :, 4 * g:4 * (g + 1), 2 * p * DH:(2 * p + 2) * DH
                        ],
                        out_sb[:].rearrange("p (s h d) -> p s h d", s=NS, h=NH)[
                            :, 4 * g:4 * (g + 1), 2 * p:2 * p + 2, :
                        ].rearrange("p s h d -> p s (h d)"),
                    )

    _split_multi_waits(
        nc, engines=("PE", "Activation", "DVE", "Pool", "SP", "GPSIMD")
    )
    nc.finalize()
    return nc


def _host_prep(query, key, vq, gq, bq, vk, gk, bk, vv, gv, bv):
    """Weight-norm folding + per-core input maps."""
    f32 = np.float32

    def wn(v, g):
        v = np.asarray(v, f32)
        g = np.asarray(g, f32)
        nrm = np.sqrt(np.sum(v * v, axis=0, dtype=f32), dtype=f32)
        return (v * (g / nrm)).astype(f32)

    wq = wn(vq, gq)
    wk = wn(vk, gk)
    wv = wn(vv, gv)
    bq_r = np.asarray(bq, f32).reshape(NCC, P).T.copy()
    bk_r = np.asarray(bk, f32).reshape(NCC, P).T.copy()
    bv_r = np.asarray(bv, f32).reshape(1, CH).copy()
    maskT = np.triu(np.ones((P, P), f32), k=1)  # maskT[k,q] = 1 iff q > k
    ident = np.eye(P, dtype=f32)

    import ml_dtypes

    bf16 = ml_dtypes.bfloat16
    query = np.asarray(query, f32)
    key = np.asarray(key, f32)
    wq_b, wk_b, wv_b = wq.astype(bf16), wk.astype(bf16), wv.astype(bf16)
    bv_b = bv_r.astype(bf16)
    mask_b = maskT.astype(bf16)
    in_maps = []
    for b in range(N_CORES):
        in_maps.append({
            "qf": np.ascontiguousarray(query[b].reshape(S, QC)).astype(bf16),
            "kf": np.ascontiguousarray(key[b].reshape(S, KC)).astype(bf16),
            "wq": wq_b, "wk": wk_b, "wv": wv_b,
            "bq": bq_r, "bk": bk_r,
            "bvb": np.broadcast_to(bv_b, (P, CH)).copy(),
            "maskT": mask_b, "ident": ident,
            "identb": ident.astype(bf16),
            "ones": np.ones((1, P), bf16),
            "bvr": bv_b.reshape(1, CH),
            "vones": np.ones((P, NH), bf16),
        })
    return in_maps


def _ensure_ntff_hook():
    """Register the axon NTFF profiling hook if the image lacks the
    antenv.axon_hooks shim module (profiling-only; no effect on results)."""
    import types

    try:
        import antenv.axon_hooks  # noqa: F401
        return
    except ImportError:
        pass
    mod = types.ModuleType("antenv.axon_hooks")
    holder = {"hook": None}
    mod.set_axon_ntff_profile_hook = lambda h: holder.__setitem__("hook", h)
    mod.get_axon_ntff_profile_hook = lambda: holder["hook"]
    sys.modules["antenv.axon_hooks"] = mod
    try:
        import antenv

        antenv.axon_hooks = mod
    except ImportError:
        pass
    try:
        from trn_agent_boot.trn_boot import _ntff_profile_via_ctypes

        mod.set_axon_ntff_profile_hook(
            _ntff_profile_via_ctypes("/opt/axon/libaxon_pjrt.so")
        )
    except Exception:
        pass


def kernel(query, key, vq, gq, bq, vk, gk, bk, vv, gv, bv):
    from concourse.bass_utils import run_bass_kernel_spmd

    global _cached_nc
    if _cached_nc is None:
        _cached_nc = build_module()
    nc = _cached_nc

    in_maps = _host_prep(query, key, vq, gq, bq, vk, gk, bk, vv, gv, bv)
    trace = os.environ.get("KERNEL_TRACE", "0") == "1"
    if trace:
        _ensure_ntff_hook()
    res = run_bass_kernel_spmd(nc, in_maps, list(range(N_CORES)), trace=trace)
    if trace and res.exec_time_ns is not None:
        print(f"HW exec time: {res.exec_time_ns} ns", flush=True)
        kernel.last_exec_time_ns = res.exec_time_ns
    out = np.stack(
        [res.results[b]["out"].reshape(32, 32, CH) for b in range(N_CORES)]
    )
    return out.astype(np.float32)


# revision 26
# speedup vs baseline: 1.2668x; 1.2668x over previous
"""Causal attention (weight-normalized projections) Trainium2 Bass kernel.

Full-input contract: kernel(**inputs) takes the unsharded tensors from
setup_inputs() and returns the full [8, 32, 32, 512] output. Internally the
batch dim (8) is sharded 1:1 across 8 NeuronCores (data parallel); each core
runs an identical Bass program on its own batch.

Math per batch b:
  qf = query[b].reshape(1024, 256); kf = key[b].reshape(1024, 512)
  q = qf @ wq + bq ; k = kf @ wk + bk ; v = kf @ wv + bv      (wx weight-normed)
  per head h (8 heads, dh=64):
    scores = q_h @ k_h.T / 8 ; strict-causal mask ; softmax ; out_h = attn @ v_h
  out[b] = concat_h(out_h).reshape(32, 32, 512)

Numerics: softmax is computed without max-subtraction (scores are ~N(0,1);
exp never overflows fp32). The mask is applied multiplicatively after exp
(0/1 mask), which matches the reference's -10000 additive mask exactly in
fp32 (exp underflows to 0). Row q=0 has an all-zero mask; its numerator and
denominator are exactly 0 and the 1e-30 epsilon makes 0/eps = 0, matching
the reference's post-softmax start-mask zeroing.
"""

import os
import sys

import numpy as np

for _p in ("/opt/trn_rl_repo", "/root/.axon_site/_ro/trn_rl_repo"):
    if _p not in sys.path and os.path.isdir(_p):
        sys.path.append(_p)

import concourse.bass as bass
import concourse.mybir as mybir
import concourse.tile as tile

FP = mybir.dt.float32
FPR = mybir.dt.float32r
BF = mybir.dt.bfloat16
AF = mybir.ActivationFunctionType


B = 8
S = 1024
QC, KC, CH = 256, 512, 512
NH, DH = 8, 64
P = 128
NS = S // P    # 8 seq chunks of 128
NAQ = QC // P  # 2 contraction chunks for q proj
NAK = KC // P  # 4 contraction chunks for k/v proj
NCC = CH // P  # 4 output-channel chunks
DH1 = DH + 1   # v columns + ones column (softmax denominator)

N_CORES = 8

_cached_nc = None


def _split_multi_waits(nc, engines=("PE",)):
    """Hoist extra sem-waits onto single-wait NoOps.

    Walrus's CoreV3 codegen rejects PE instructions carrying more than one
    sync wait (setupSyncWait<S3_LW_STRUCT>: "Too many sync wait commands").
    Tile's scheduler freely attaches several waits to one instruction, so
    after scheduling we move all but the last wait of each affected
    instruction onto dedicated same-engine NoOps placed directly before it;
    the engine's sequencer blocks on each NoOp in program order, preserving
    semantics exactly.
    """
    ctr = 0
    for fn in nc.m.functions:
        for blk in fn.blocks:
            new_insts = []
            for inst in blk.instructions:
                si = getattr(inst, "sync_info", None)
                waits = list(si.on_wait) if si is not None and si.on_wait else []
                eng = getattr(inst, "engine", None)
                if (
                    len(waits) > 1
                    and eng is not None
                    and any(e in str(eng) for e in engines)
                ):
                    for w in waits[:-1]:
                        nop = mybir.InstNoOp(
                            name=f"I-wsplit-{ctr}",
                            engine=eng,
                            sync_info=mybir.SyncInfo(on_wait=[w], on_update=[]),
                            bass_nofuse=True,
                        )
                        ctr += 1
                        new_insts.append(nop)
                        nc.inst_map[nop.name] = nop
                    inst.sync_info = mybir.SyncInfo(
                        on_wait=[waits[-1]],
                        on_update=list(si.on_update) if si.on_update else [],
                    )
                new_insts.append(inst)
            blk.instructions[:] = new_insts


def build_module() -> "bass.Bass":
    nc = bass.Bass()

    qf_d = nc.dram_tensor("qf", [S, QC], BF, kind="ExternalInput")
    kf_d = nc.dram_tensor("kf", [S, KC], BF, kind="ExternalInput")
    wq_d = nc.dram_tensor("wq", [QC, CH], BF, kind="ExternalInput")
    wk_d = nc.dram_tensor("wk", [KC, CH], BF, kind="ExternalInput")
    wv_d = nc.dram_tensor("wv", [KC, CH], BF, kind="ExternalInput")
    bq_d = nc.dram_tensor("bq", [P, NCC], FP, kind="ExternalInput")
    bk_d = nc.dram_tensor("bk", [P, NCC], FP, kind="ExternalInput")
    bvb_d = nc.dram_tensor("bvb", [P, CH], BF, kind="ExternalInput")
    mask_d = nc.dram_tensor("maskT", [P, P], BF, kind="ExternalInput")
    id_d = nc.dram_tensor("ident", [P, P], FP, kind="ExternalInput")
    idb_d = nc.dram_tensor("identb", [P, P], BF, kind="ExternalInput")
    ones_d = nc.dram_tensor("ones", [1, P], BF, kind="ExternalInput")
    bvr_d = nc.dram_tensor("bvr", [1, CH], BF, kind="ExternalInput")
    vones_d = nc.dram_tensor("vones", [P, NH], BF, kind="ExternalInput")
    out_d = nc.dram_tensor("out", [S, CH], FP, kind="ExternalOutput")

    QW = 512  # q-half width

    with tile.TileContext(nc) as tc:
        with (
            tc.tile_pool(name="const", bufs=1) as cpool,
            tc.tile_pool(name="work", bufs=2) as wpool,
            tc.tile_pool(name="psA", bufs=2, space=bass.MemorySpace.PSUM) as psA,
            tc.tile_pool(name="psB", bufs=3, space=bass.MemorySpace.PSUM) as psB,
            tc.tile_pool(name="psC", bufs=1, space=bass.MemorySpace.PSUM) as psC,
        ):
            # ---- inputs: natural contiguous loads; transpose on (idle) PE ----
            qfT = [cpool.tile([P, S], BF, tag=f"qfT{a}", name=f"qfT{a}") for a in range(NAQ)]
            kfT = [cpool.tile([P, S], BF, tag=f"kfT{a}", name=f"kfT{a}") for a in range(NAK)]
            wq_sb = [cpool.tile([P, CH], BF, tag=f"wq{a}", name=f"wq{a}") for a in range(NAQ)]
            wk_sb = [cpool.tile([P, CH], BF, tag=f"wk{a}", name=f"wk{a}") for a in range(NAK)]
            wv_sb = [cpool.tile([P, CH], BF, tag=f"wv{a}", name=f"wv{a}") for a in range(NAK)]
            idb_sb = cpool.tile([P, P], BF, tag="identb", name="idb_sb")
            nc.sync.dma_start(idb_sb[:], idb_d[:])
            # PE warm-up: dense dummy matmuls during the input-DMA window keep
            # the HAM activity monitor busy so projections start at 2.4 GHz
            # instead of the cold 1.2 GHz half-clock.
            warm_ps = psC.tile([P, QW], FP, tag="tp", name="warm_ps")
            for _w in range(56):
                nc.tensor.matmul(
                    warm_ps[0:P, 0:P], idb_sb[:], idb_sb[:],
                    start=True, stop=True,
                )
            qf_sb = [cpool.tile([P, QC], BF, tag=f"qf{si}", name=f"qf{si}") for si in range(NS)]
            kf_sb = [cpool.tile([P, KC], BF, tag=f"kf{si}", name=f"kf{si}") for si in range(NS)]
            for si in range(NS):
                nc.sync.dma_start(qf_sb[si][:], qf_d[si * P:(si + 1) * P, :])
            for a in range(NAQ):
                nc.sync.dma_start(wq_sb[a][:], wq_d[a * P:(a + 1) * P, :])
            for si in range(NS):
                nc.sync.dma_start(kf_sb[si][:], kf_d[si * P:(si + 1) * P, :])
            for a in range(NAK):
                nc.sync.dma_start(wk_sb[a][:], wk_d[a * P:(a + 1) * P, :])
                nc.sync.dma_start(wv_sb[a][:], wv_d[a * P:(a + 1) * P, :])
            bq_sb = cpool.tile([P, NCC], FP, tag="bq", name="bq_sb")
            bk_sb = cpool.tile([P, NCC], FP, tag="bk", name="bk_sb")
            bvb_sb = cpool.tile([P, CH], BF, tag="bvb", name="bvb_sb")
            ones_sb = cpool.tile([1, P], BF, tag="ones", name="ones_sb")
            bvr_sb = cpool.tile([1, CH], BF, tag="bvr", name="bvr_sb")
            nc.sync.dma_start(ones_sb[:], ones_d[:])
            nc.sync.dma_start(bvr_sb[:], bvr_d[:])
            mask_sb = cpool.tile([P, P], BF, tag="mask", name="mask_sb")
            id_sb = cpool.tile([P, P], FP, tag="ident", name="id_sb")
            nc.sync.dma_start(bq_sb[:], bq_d[:])
            nc.sync.dma_start(bk_sb[:], bk_d[:])
            nc.sync.dma_start(bvb_sb[:], bvb_d[:])
            nc.sync.dma_start(mask_sb[:], mask_d[:])
            nc.sync.dma_start(id_sb[:], id_d[:])

            for a in range(NAQ):
                ps = psA.tile([P, S], FP, tag="sc", name="sc_ps")
                for si in range(NS):
                    nc.tensor.matmul(
                        ps[:, si * P:(si + 1) * P],
                        qf_sb[si][:, a * P:(a + 1) * P],
                        idb_sb[:],
                        start=True,
                        stop=True,
                    )
                if a % 2 == 0:
                    nc.vector.tensor_copy(qfT[a][:], ps[:])
                else:
                    nc.scalar.copy(qfT[a][:], ps[:])
            for a in range(NAK):
                ps = psA.tile([P, S], FP, tag="sc", name="sc_ps")
                for si in range(NS):
                    nc.tensor.matmul(
                        ps[:, si * P:(si + 1) * P],
                        kf_sb[si][:, a * P:(a + 1) * P],
                        idb_sb[:],
                        start=True,
                        stop=True,
                    )
                if a % 2 == 0:
                    nc.vector.tensor_copy(kfT[a][:], ps[:])
                else:
                    nc.scalar.copy(kfT[a][:], ps[:])

            # ---------------- projections ----------------
            # qT/kT in [channel, seq] layout (head-dim on partitions)
            qT = [cpool.tile([P, S], BF, tag=f"qT{c}", name=f"qT{c}") for c in range(NCC)]
            kT = [cpool.tile([P, S], BF, tag=f"kT{c}", name=f"kT{c}") for c in range(NCC)]

            def emit_qT(c):
                ps = psA.tile([P, S], FP, tag="sc", name="sc_ps")
                for a in range(NAQ):
                    for g in range(2):
                        nc.tensor.matmul(
                            ps[:, g * QW:(g + 1) * QW],
                            wq_sb[a][:, c * P:(c + 1) * P],
                            qfT[a][:, g * QW:(g + 1) * QW],
                            start=(a == 0),
                            stop=(a == NAQ - 1),
                        )
                nc.scalar.activation(
                    qT[c][:], ps[:], AF.Identity, bias=bq_sb[:, c:c + 1]
                )

            def emit_kT(c):
                ps = psA.tile([P, S], FP, tag="sc", name="sc_ps")
                for a in range(NAK):
                    for g in range(2):
                        nc.tensor.matmul(
                            ps[:, g * QW:(g + 1) * QW],
                            wk_sb[a][:, c * P:(c + 1) * P],
                            kfT[a][:, g * QW:(g + 1) * QW],
                            start=(a == 0),
                            stop=(a == NAK - 1),
                        )
                nc.scalar.activation(
                    kT[c][:], ps[:], AF.Identity, bias=bk_sb[:, c:c + 1]
                )

            # v[s, c] per-head blocks of 65 cols (64 data + ones col for the
            # softmax denominator); bias added on DVE during evacuation
            v_sb = [cpool.tile([P, NH * DH1], BF, tag=f"v{si}", name=f"v{si}") for si in range(NS)]
            bvb_view = bvb_sb[:].rearrange("p (h d) -> p h d", h=NH)

            def emit_v(si):
                ps = psA.tile([P, S], FP, tag="sc", name="sc_ps")
                for a in range(NAK):
                    nc.tensor.matmul(
                        ps[:, 0:CH],
                        kfT[a][:, si * P:(si + 1) * P],
                        wv_sb[a][:],
                        start=(a == 0),
                        stop=False,
                    )
                nc.tensor.matmul(
                    ps[:, 0:CH], ones_sb[:], bvr_sb[:], start=False, stop=True
                )
                v_view = v_sb[si][:].rearrange("p (h d) -> p h d", h=NH)
                nc.vector.tensor_copy(
                    v_view[:, :, 0:DH],
                    ps[:, 0:CH].rearrange("p (h d) -> p h d", h=NH),
                )
                nc.sync.dma_start(
                    v_view[:, :, DH:DH1],
                    vones_d[:].rearrange("p (h o) -> p h o", o=1),
                )

            for c in range(NCC):
                emit_qT(c)
            for c in range(NCC):
                emit_kT(c)
            for si in range(NS):
                emit_v(si)

            # ---------------- attention: head pairs x q-halves ----------------
            # Heads 2p/2p+1 share qT[p]/kT[p] (rows 0:64 / 64:128). QK for the
            # two heads is row-packed onto the PE array (tile_position), the
            # exp over both heads' scores is one ACT instruction, and the two
            # AV chains interleave to keep PE fed while ACT runs.
            out_sb = cpool.tile([P, NS * CH], FP, tag="osb", name="out_sb")
            mask_b2 = mask_sb[:].rearrange("p (o w) -> p o w", o=1).broadcast_to((P, 2, P))

            for p in range(NH // 2):
                tq = qT[p]
                tk = kT[p]
                v_hp = [
                    [v_sb[j][:].rearrange("p (h d) -> p h d", h=NH)[:, 2 * p + idx, :]
                     for idx in range(2)]
                    for j in range(NS)
                ]
                for g in range(2):
                    jmax = 4 * (g + 1)
                    outp = [
                        psB.tile([P, QW], FP, tag="outp", name="outp_ps")
                        for _ in range(2)
                    ]

                    def emit_qk(j):
                        off = max(0, j * P - g * QW)
                        sc = psA.tile([P, 2 * QW], FP, tag="sc", name="sc_ps")
                        for idx in range(2):
                            nc.tensor.matmul(
                                sc[:, idx * QW + off:(idx + 1) * QW],
                                tk[idx * DH:(idx + 1) * DH, j * P:(j + 1) * P],
                                tq[idx * DH:(idx + 1) * DH, g * QW + off:(g + 1) * QW],
                                start=True,
                                stop=True,
                                tile_position=(idx * DH, 0),
                            )
                        ex = wpool.tile([P, 2 * QW], BF, tag="ex", name="ex_t", bufs=3)
                        scv = sc[:].rearrange("p (i w) -> p i w", i=2)[:, :, off:QW]
                        exv = ex[:].rearrange("p (i w) -> p i w", i=2)[:, :, off:QW]
                        nc.scalar.activation(exv, scv, AF.Exp, scale=0.125)
                        if g * 4 <= j < g * 4 + 4:  # diagonal block in this half
                            od = j * P - g * QW
                            exd = ex[:].rearrange("p (i w) -> p i w", i=2)[:, :, od:od + P]
                            nc.vector.tensor_mul(exd, exd, mask_b2)
                        return ex

                    def emit_av(j, ex):
                        off = max(0, j * P - g * QW)
                        for idx in range(2):
                            nc.tensor.matmul(
                                outp[idx][0:DH1, off:QW],
                                v_hp[j][idx],
                                ex[:, idx * QW + off:(idx + 1) * QW],
                                start=(j == 0),
                                stop=(j == jmax - 1),
                                skip_group_check=True,
                            )

                    prev_ex = emit_qk(0)
                    for j in range(1, jmax):
                        cur_ex = emit_qk(j)
                        emit_av(j - 1, prev_ex)
                        prev_ex = cur_ex
                    emit_av(jmax - 1, prev_ex)

                    # epilogue per head: evac, transpose to [q, d], normalize
                    for idx in range(2):
                        h = 2 * p + idx
                        outs = wpool.tile([P, QW], FP, tag="outs", name="outs_t")
                        nc.vector.tensor_copy(outs[0:DH1, :], outp[idx][0:DH1, :])
                        tp2 = psC.tile([P, QW], FP, tag="tp", name="tp_ps")
                        for ls in range(4):
                            nc.tensor.transpose(
                                tp2[:, ls * P:ls * P + DH1],
                                outs[0:DH1, ls * P:(ls + 1) * P],
                                id_sb[0:DH1, 0:DH1],
                            )
                        tpv = tp2[:].rearrange("p (s c) -> p s c", c=P)
                        rc = wpool.tile([P, 4], FP, tag="rc", name="rc_t")
                        rc2 = wpool.tile([P, 4], FP, tag="rc2", name="rc2_t")
                        nc.vector.tensor_scalar_add(rc[:], tpv[:, :, DH:DH1], 1e-30)
                        nc.vector.reciprocal(rc2[:], rc[:])
                        out_view = out_sb[:].rearrange(
                            "p (s h d) -> p s h d", s=NS, h=NH
                        )[:, 4 * g:4 * (g + 1), h, :]
                        rc_b = rc2[:].rearrange("p (s o) -> p s o", o=1).broadcast_to(
                            (P, 4, DH)
                        )
                        nc.vector.tensor_mul(out_view, tpv[:, :, 0:DH], rc_b)

                    if p == 0 and g == 0:
                        # row q=0 is exactly zero (start mask); write before
                        # the first partial output DMA of chunk (p=0, g=0)
                        nc.vector.memset(
                            out_sb[0:1, 0:2 * DH], 0.0
                        )
                    nc.sync.dma_start(
                        out_d.rearrange("(s p) c -> p s c", p=P)[
                            :, 4 * g:4 * (g + 1), 2 * p * DH:(2 * p + 2) * DH
                        ],
                        out_sb[:].rearrange("p (s h d) -> p s h d", s=NS, h=NH)[
                            :, 4 * g:4 * (g + 1), 2 * p:2 * p + 2, :
                        ].rearrange("p s h d -> p s (h d)"),
                    )

    _split_multi_waits(
        nc, engines=("PE", "Activation", "DVE", "Pool", "SP", "GPSIMD")
    )
    nc.finalize()
    return nc


def _host_prep(query, key, vq, gq, bq, vk, gk, bk, vv, gv, bv):
    """Weight-norm folding + per-core input maps."""
    f32 = np.float32

    def wn(v, g):
        v = np.asarray(v, f32)
        g = np.asarray(g, f32)
        nrm = np.sqrt(np.sum(v * v, axis=0, dtype=f32), dtype=f32)
        return (v * (g / nrm)).astype(f32)

    wq = wn(vq, gq)
    wk = wn(vk, gk)
    wv = wn(vv, gv)
    bq_r = np.asarray(bq, f32).reshape(NCC, P).T.copy()
    bk_r = np.asarray(bk, f32).reshape(NCC, P).T.copy()
    bv_r = np.asarray(bv, f32).reshape(1, CH).copy()
    maskT = np.triu(np.ones((P, P), f32), k=1)  # maskT[k,q] = 1 iff q > k
    ident = np.eye(P, dtype=f32)

    import ml_dtypes

    bf16 = ml_dtypes.bfloat16
    query = np.asarray(query, f32)
    key = np.asarray(key, f32)
    wq_b, wk_b, wv_b = wq.astype(bf16), wk.astype(bf16), wv.astype(bf16)
    bv_b = bv_r.astype(bf16)
    mask_b = maskT.astype(bf16)
    in_maps = []
    for b in range(N_CORES):
        in_maps.append({
            "qf": np.ascontiguousarray(query[b].reshape(S, QC)).astype(bf16),
            "kf": np.ascontiguousarray(key[b].reshape(S, KC)).astype(bf16),
            "wq": wq_b, "wk": wk_b, "wv": wv_b,
            "bq": bq_r, "bk": bk_r,
            "bvb": np.broadcast_to(bv_b, (P, CH)).copy(),
            "maskT": mask_b, "ident": ident,
            "identb": ident.astype(bf16),
            "ones": np.ones((1, P), bf16),
            "bvr": bv_b.reshape(1, CH),
            "vones": np.ones((P, NH), bf16),
        })
    return in_maps


def _ensure_ntff_hook():
    """Register the axon NTFF profiling hook if the image lacks the
    antenv.axon_hooks shim module (profiling-only; no effect on results)."""
    import types

    try:
        import antenv.axon_hooks  # noqa: F401
        return
    except ImportError:
        pass
    mod = types.ModuleType("antenv.axon_hooks")
    holder = {"hook": None}
    mod.set_axon_ntff_profile_hook = lambda h: holder.__setitem__("hook", h)
    mod.get_axon_ntff_profile_hook = lambda: holder["hook"]
    sys.modules["antenv.axon_hooks"] = mod
    try:
        import antenv

        antenv.axon_hooks = mod
    except ImportError:
        pass
    try:
        from trn_agent_boot.trn_boot import _ntff_profile_via_ctypes

        mod.set_axon_ntff_profile_hook(
            _ntff_profile_via_ctypes("/opt/axon/libaxon_pjrt.so")
        )
    except Exception:
        pass


def kernel(query, key, vq, gq, bq, vk, gk, bk, vv, gv, bv):
    from concourse.bass_utils import run_bass_kernel_spmd

    global _cached_nc
    if _cached_nc is None:
        _cached_nc = build_module()
    nc = _cached_nc

    in_maps = _host_prep(query, key, vq, gq, bq, vk, gk, bk, vv, gv, bv)
    trace = os.environ.get("KERNEL_TRACE", "0") == "1"
    if trace:
        _ensure_ntff_hook()
    res = run_bass_kernel_spmd(nc, in_maps, list(range(N_CORES)), trace=trace)
    if trace and res.exec_time_ns is not None:
        print(f"HW exec time: {res.exec_time_ns} ns", flush=True)
        kernel.last_exec_time_ns = res.exec_time_ns
    out = np.stack(
        [res.results[b]["out"].reshape(32, 32, CH) for b in range(N_CORES)]
    )
    return out.astype(np.float32)
